# revision 14
# baseline (speedup 1.0000x reference)
"""Segment softmax (GAT attention stage 4) on 8 TRN2 NeuronCores.

alpha_i = exp(e_i) / sum_{j: tgt_j == tgt_i} exp(e_j) — identical to the
reference: with e ~ N(0,1) the max-shift cancels exactly and the 1e-16
regularizer is negligible (every segment is non-empty w.o.p.).

Strategy: shard NODES across the 8 cores (each target node's edges live on
exactly one core), so there is no cross-core reduction at all. The host
ranks nodes by degree, deals them round-robin to cores (identical degree
profile per core), and packs each node's edges into a contiguous column
range of one SBUF partition row; 128 node positions form a "chunk", chunks
with similar capacity form uniform-[128, G, C] "slabs" (~3% padding).

v2 changes vs the 60-70us baseline (trace-driven):
  - Edge logits ship as symmetric int8 (q = max|e|/127); ACT fuses the
    dequant into the table op: x = Exp(q * u). The missing zero-offset
    cancels in the softmax. Halves input HBM traffic; measured end-to-end
    rel_l2 ~1.2e-2 vs the 2e-2 gate on the seeded data. Pad byte -128
    gives exp ~ 4e-3, adding < 3e-4 relative to any segment sum, and
    keeps empty-lane sums positive (no eps pass needed).
  - The ~98 per-chunk normalize multiplies (24us DVE + 24us ACT in the
    v1 trace) are replaced by ONE slab-wide DVE tensor_tensor with a
    stride-0 (broadcast) AP on the reciprocal operand, plus a few
    per-chunk ACT Copy+scale ops for engine balance.
  - Three fold-add levels (C%8) ahead of the grouped tensor_reduce.
  - Output alpha returns as fp16.
"""

import numpy as np

P = 128
NCORES = 8
S_MAX = 6144  # max columns per slab (per partition)
G_MAX = 192  # max chunks per core

# measured per-op costs, ns (fixed, per-column)
ACT_EXP = (350.0, 0.88)
ACT_MUL = (600.0, 0.0)  # ~flat for C in [190, 350]
DVE_TT = (140.0, 0.52)  # tensor_tensor fp16 packed (2x) per output column
DVE_BM = (150.0, 0.547)  # pair-broadcast tensor_tensor (2x)
DVE_RED = (170.0, 1.24)  # grouped tensor_reduce per input column
DVE_FIX = 200.0

_CACHE = {}


def _plan(deg, num_nodes):
    """Node ranking, chunk capacities, slab grouping. Data-dependent."""
    N = num_nodes
    npc = -(-N // NCORES)  # node positions per core
    G = -(-npc // P)  # chunks per core
    order = np.argsort(-deg, kind="stable")
    deg_sorted = deg[order].astype(np.int64)
    caps = np.empty(G, dtype=np.int64)
    for g in range(G):
        lead = min(g * P * NCORES, N - 1)
        caps[g] = max(1, int(deg_sorted[lead]))
    smax = max(S_MAX, int(-(-int(caps[0]) // 8) * 8))
    slabs = []  # (col_off, G_s, C_s)
    g = 0
    off = 0
    while g < G:
        C_s = -(-int(caps[g]) // 8) * 8  # %8 for the three fold levels
        G_s = 1
        while (
            g + G_s < G
            and (G_s + 1) * C_s <= smax
            and (C_s - caps[g + G_s]) <= max(8, C_s // 16)
        ):
            G_s += 1
        slabs.append((off, G_s, C_s))
        off += G_s * C_s
        g += G_s
    W = off
    chunk_off = np.empty(G, dtype=np.int64)
    g = 0
    for s_off, G_s, C_s in slabs:
        for k in range(G_s):
            chunk_off[g] = s_off + k * C_s
            g += 1
    return order, deg_sorted, slabs, W, chunk_off


def _order(slabs):
    """Device processing order: fully ascending sizes. With a deep exp
    prefetch the per-slab DVE work (~1.1 ns/col) outpaces the next exp
    (~0.9 ns/col), so an ascending ramp never starves the DVE after the
    first slab; the tail is handled by splitting the last slab's
    normalize + output DMA in half."""
    return sorted(range(len(slabs)), key=lambda s: slabs[s][1] * slabs[s][2])


def _mul_split(slabs, proc):
    """Per-slab n_act: the first n_act chunks normalize on ACT (per-chunk
    Copy+scale), the rest in one pair-broadcast DVE tensor_tensor. Greedy
    balance of predicted engine loads; the last two processed slabs stay
    fully on DVE so the ACT queue drains early (short tail)."""
    total_cols = sum(G_s * C_s for _, G_s, C_s in slabs)
    ns = len(slabs)
    act = ns * ACT_EXP[0] + total_cols * ACT_EXP[1]
    # folds (3 levels) + grouped reduce + recip/dup-cast + 2x bmul
    dve = (
        total_cols * (0.5 + 0.25 + 0.125) * DVE_TT[1]
        + total_cols * 0.125 * DVE_RED[1]
        + ns * (3 * DVE_TT[0] + DVE_RED[0] + 2 * DVE_FIX + DVE_BM[0])
        + total_cols * DVE_BM[1]
    )
    n_act = [0] * ns
    late = set(proc[-2:]) if len(proc) > 3 else set()
    idx = [s for s in sorted(range(ns), key=lambda s: -slabs[s][2]) if s not in late]
    moved = True
    while moved and idx:
        moved = False
        for s in idx:
            _, G_s, C_s = slabs[s]
            if n_act[s] >= G_s - 1:
                continue
            d_dve = C_s * DVE_BM[1]
            d_act = ACT_MUL[0] + C_s * ACT_MUL[1]
            # bias toward ACT: the ACT queue drains earlier than the DVE
            # (no ACT work in the tail), so overshoot its static load
            if dve - act > d_dve + d_act - 2500.0:
                n_act[s] += 1
                dve -= d_dve
                act += d_act
                moved = True
    return n_act


def _build(slabs, W, q):
    import concourse.mybir as mybir
    from concourse import bacc
    from concourse.tile import TileContext

    nc = bacc.Bacc(None, target_bir_lowering=False)
    ev = nc.dram_tensor("ev", [P, W], mybir.dt.int8, kind="ExternalInput")
    av = nc.dram_tensor("av", [P, W], mybir.dt.float16, kind="ExternalOutput")

    smax = max(S_MAX, max(C for _, _, C in slabs))
    proc = _order(slabs)
    n_acts = _mul_split(slabs, proc)
    with TileContext(nc) as tc:
        with tc.tile_pool(name="sbuf", bufs=4) as pool:

            def load_exp(si):
                """Stage 1: input DMA + slab-wide dequantizing exp."""
                off, G_s, C_s = slabs[si]
                S = G_s * C_s
                et = pool.tile([P, smax], mybir.dt.int8, tag="e")
                nc.sync.dma_start(out=et[:, :S], in_=ev[:, off : off + S])
                xt = pool.tile([P, smax], mybir.dt.float16, tag="x")
                nc.scalar.activation(
                    xt[:, :S],
                    et[:, :S],
                    mybir.ActivationFunctionType.Exp,
                    scale=float(q),
                )
                return xt

            def normalize(si, xt, split_tail=False):
                """Stage 2: fold-adds, grouped reduce, recip, multiplies,
                output DMA. Emitted behind load_exp so the ACT queue
                always has upcoming exps ahead of this slab's per-chunk
                multiplies (keeps the DVE fed). split_tail halves the
                final multiply + output DMA so the last transfer overlaps
                the last compute."""
                (off, G_s, C_s), n_act = slabs[si], n_acts[si]
                S = G_s * C_s
                st = pool.tile([P, G_MAX], mybir.dt.float32, tag="s")
                x3 = xt[:, :S].rearrange("p (g c) -> p g c", g=G_s)
                if C_s % 8 == 0 and C_s >= 32:
                    h, hq, he = C_s // 2, C_s // 4, C_s // 8
                    yt = pool.tile([P, smax // 2], mybir.dt.float16, tag="y")
                    y3 = yt[:, : G_s * h].rearrange("p (g c) -> p g c", g=G_s)
                    nc.vector.tensor_add(out=y3, in0=x3[:, :, :h], in1=x3[:, :, h:])
                    zt = pool.tile([P, smax // 4], mybir.dt.float16, tag="z")
                    z3 = zt[:, : G_s * hq].rearrange("p (g c) -> p g c", g=G_s)
                    nc.vector.tensor_add(out=z3, in0=y3[:, :, :hq], in1=y3[:, :, hq:])
                    wt = pool.tile([P, smax // 8], mybir.dt.float16, tag="w")
                    w3 = wt[:, : G_s * he].rearrange("p (g c) -> p g c", g=G_s)
                    nc.vector.tensor_add(out=w3, in0=z3[:, :, :he], in1=z3[:, :, he:])
                    red_in = w3
                elif C_s % 4 == 0 and C_s >= 8:
                    h, hq = C_s // 2, C_s // 4
                    yt = pool.tile([P, smax // 2], mybir.dt.float16, tag="y")
                    y3 = yt[:, : G_s * h].rearrange("p (g c) -> p g c", g=G_s)
                    nc.vector.tensor_add(out=y3, in0=x3[:, :, :h], in1=x3[:, :, h:])
                    zt = pool.tile([P, smax // 4], mybir.dt.float16, tag="z")
                    z3 = zt[:, : G_s * hq].rearrange("p (g c) -> p g c", g=G_s)
                    nc.vector.tensor_add(out=z3, in0=y3[:, :, :hq], in1=y3[:, :, hq:])
                    red_in = z3
                else:
                    red_in = x3
                nc.vector.tensor_reduce(
                    out=st[:, :G_s],
                    in_=red_in,
                    axis=mybir.AxisListType.X,
                    op=mybir.AluOpType.add,
                )
                qt32 = pool.tile([P, G_MAX], mybir.dt.float32, tag="q32")
                nc.vector.reciprocal(out=qt32[:, :G_s], in_=st[:, :G_s])
                # duplicated-pair fp16 reciprocals: the bmul broadcast AP
                # gets a packed (stride-1, count-2) last dim, which keeps
                # the DVE in 2x mode (a plain stride-0 operand drops to 1x)
                qt2 = pool.tile([P, 2 * G_MAX], mybir.dt.float16, tag="q2")
                with nc.allow_low_precision(reason="1/s fits fp16; gate 2e-2"):
                    nc.vector.tensor_copy(
                        out=qt2[:, : 2 * G_s].rearrange("p (g i) -> p g i", g=G_s),
                        in_=qt32[:, :G_s]
                        .rearrange("p g -> p g ()")
                        .to_broadcast([P, G_s, 2]),
                    )
                at = pool.tile([P, smax], mybir.dt.float16, tag="a")
                for g in range(n_act):
                    o = slice(g * C_s, (g + 1) * C_s)
                    nc.scalar.mul(at[:, o], xt[:, o], qt32[:, g : g + 1])

                def bmul(g0, g1):
                    Gr = g1 - g0
                    o = slice(g0 * C_s, g1 * C_s)
                    nc.vector.tensor_mul(
                        out=at[:, o].rearrange(
                            "p (g c2 i) -> p g c2 i", g=Gr, i=2
                        ),
                        in0=xt[:, o].rearrange(
                            "p (g c2 i) -> p g c2 i", g=Gr, i=2
                        ),
                        in1=qt2[:, 2 * g0 : 2 * g1]
                        .rearrange("p (g i) -> p g () i", g=Gr)
                        .to_broadcast([P, Gr, C_s // 2, 2]),
                    )

                if split_tail and G_s - n_act >= 2:
                    gm = n_act + (G_s - n_act) // 2
                    bmul(n_act, gm)
                    nc.sync.dma_start(
                        out=av[:, off : off + gm * C_s], in_=at[:, : gm * C_s]
                    )
                    bmul(gm, G_s)
                    nc.sync.dma_start(
                        out=av[:, off + gm * C_s : off + S],
                        in_=at[:, gm * C_s : S],
                    )
                else:
                    if n_act < G_s:
                        bmul(n_act, G_s)
                    nc.sync.dma_start(out=av[:, off : off + S], in_=at[:, :S])

            pending = []
            for si in proc:
                xt = load_exp(si)
                pending.append((si, xt))
                if len(pending) > 3:  # 3-deep exp prefetch
                    normalize(*pending.pop(0))
            for k, item in enumerate(pending):
                normalize(*item, split_tail=(k == len(pending) - 1))
    nc.compile()
    return nc


def _prepare(e, tgt, num_nodes):
    """Host-side pack: (per-core int8 arrays, scale, scatter metadata)."""
    E = e.shape[0]
    N = num_nodes
    deg = np.bincount(tgt, minlength=N).astype(np.int64)
    order, deg_sorted, slabs, W, chunk_off = _plan(deg, N)

    q = float(np.abs(e).max()) / 127.0
    e8 = np.clip(np.rint(e * (1.0 / q)), -127, 127).astype(np.int8)

    rankpos = np.empty(N, dtype=np.int64)
    rankpos[order] = np.arange(N, dtype=np.int64)
    r = rankpos[tgt]  # [E] degree-rank of each edge's target
    sidx = np.argsort(r, kind="stable")  # edges grouped by rank
    rs = r[sidx]
    starts = np.concatenate(([0], np.cumsum(deg_sorted[:-1])))
    j = np.arange(E, dtype=np.int64) - starts[rs]  # slot within node
    core = rs % NCORES
    pos = rs // NCORES
    gidx = pos // P
    lane = pos % P
    col = chunk_off[gidx] + j
    flat = lane * W + col

    ev = np.full((NCORES, P * W), -128, dtype=np.int8)
    ev[core, flat] = e8[sidx]
    return ev, slabs, W, q, sidx, core, flat


def kernel(e, edge_index, num_nodes):
    from concourse.bass_utils import run_bass_kernel_spmd

    e = np.ascontiguousarray(np.asarray(e, dtype=np.float32))
    tgt = np.asarray(edge_index)[1].astype(np.int64)
    N = int(num_nodes)
    E = e.shape[0]

    ev, slabs, W, q, sidx, core, flat = _prepare(e, tgt, N)

    key = (tuple(slabs), W, round(q, 9))
    if key not in _CACHE:
        _CACHE[key] = _build(slabs, W, q)
    nc = _CACHE[key]

    in_maps = [{"ev": ev[c].reshape(P, W)} for c in range(NCORES)]
    res = run_bass_kernel_spmd(nc, in_maps, core_ids=list(range(NCORES)))

    av = np.stack([res.results[c]["av"].reshape(-1) for c in range(NCORES)])
    alpha = np.empty(E, dtype=np.float32)
    alpha[sidx] = av[core, flat].astype(np.float32)
    return alpha


# revision 18
# speedup vs baseline: 1.1406x; 1.1406x over previous
"""Segment softmax (GAT attention stage 4) on 8 TRN2 NeuronCores.

alpha_i = exp(e_i) / sum_{j: tgt_j == tgt_i} exp(e_j) — identical to the
reference: with e ~ N(0,1) the max-shift cancels exactly and the 1e-16
regularizer is negligible (every segment is non-empty w.o.p.).

Strategy: shard NODES across the 8 cores (each target node's edges live on
exactly one core), so there is no cross-core reduction at all. The host
ranks nodes by degree, deals them round-robin to cores (identical degree
profile per core), and packs each node's edges into a contiguous column
range of one SBUF partition row; 128 node positions form a "chunk", chunks
with similar capacity form uniform-[128, G, C] "slabs" (~3% padding).

Trace-driven changes vs the 60-70us v1 baseline:
  - Edge logits ship as symmetric int8 (q = max|e|/127); ACT fuses the
    dequant into the table op: x = Exp(q * u). The missing zero-offset
    cancels in the softmax. Halves input HBM traffic; measured end-to-end
    rel_l2 ~1.2e-2 vs the 2e-2 gate on the seeded data. Pad byte -128
    gives exp ~ 4e-3, adding < 3e-4 relative to any segment sum, and
    keeps empty-lane sums positive (no eps pass needed).
  - The ~98 per-chunk normalize multiplies (24us DVE + 24us ACT in the
    v1 trace) are replaced by ONE slab-wide DVE tensor_tensor against a
    broadcast AP of the per-chunk reciprocals. The reciprocals are
    stored as duplicated PAIRS so the broadcast AP ends in a packed
    (stride-1, count-2) dim: a plain stride-0 operand drops the DVE to
    1x (0.88 ns/col measured), the pair layout keeps 2x (0.55 ns/col).
    A few per-chunk ACT Copy+scale ops take the residual for balance.
  - Three fold-add levels (C%8, fp16 tensor_tensor at 2x) ahead of the
    grouped tensor_reduce (which runs ~1.25 ns/col regardless of dtype).
  - Two-stage software pipeline: each slab's DMA+exp is emitted two
    slabs ahead of its folds/normalize so the ACT queue never blocks
    the next exp behind per-chunk multiplies (which wait on the DVE
    reciprocal); processing order tiny, mid, then descending keeps both
    the ramp and the tail short; the last slab's multiply + output DMA
    are split in half to overlap the final transfer.
  - Output alpha returns as fp16.

Engine budget per core (measured): ACT ~35us (exp 23 + muls), DVE ~35us
(folds 15.5 + bmul 12 + reduce 4.6 + recip/dup 3), DMA ~25us active,
preamble ~10us (iram fetch + engine table loads, framework-fixed) and
drain tail ~4us. PE/GPSIMD offload was evaluated and rejected: matmul
reduces across partitions only (segments lie along the free dim; a
transposed layout makes the normalize operand partition-varying, which
the lane-locked DVE cannot broadcast, and PSUM operands force 1x), and
GPSIMD tensor ops measure ~7x slower with SBUF port contention.
"""

import numpy as np

P = 128
NCORES = 8
S_MAX = 6144  # max columns per slab (per partition)
G_MAX = 192  # max chunks per core

# measured per-op costs, ns (fixed, per-column)
ACT_EXP = (350.0, 0.88)
ACT_MUL = (600.0, 0.0)  # ~flat for C in [190, 350]
DVE_TT = (140.0, 0.52)  # tensor_tensor fp16 packed (2x) per output column
DVE_BM = (150.0, 0.547)  # pair-broadcast tensor_tensor (2x)
DVE_RED = (170.0, 1.24)  # grouped tensor_reduce per input column
DVE_FIX = 200.0

_CACHE = {}


def _plan(deg, num_nodes):
    """Node ranking, chunk capacities, slab grouping. Data-dependent."""
    N = num_nodes
    npc = -(-N // NCORES)  # node positions per core
    G = -(-npc // P)  # chunks per core
    order = np.argsort(-deg, kind="stable")
    deg_sorted = deg[order].astype(np.int64)
    caps = np.empty(G, dtype=np.int64)
    for g in range(G):
        lead = min(g * P * NCORES, N - 1)
        caps[g] = max(1, int(deg_sorted[lead]))
    smax = max(S_MAX, int(-(-int(caps[0]) // 8) * 8))
    slabs = []  # (col_off, G_s, C_s)
    g = 0
    off = 0
    while g < G:
        C_s = -(-int(caps[g]) // 8) * 8  # %8 for the three fold levels
        G_s = 1
        while (
            g + G_s < G
            and (G_s + 1) * C_s <= smax
            and (C_s - caps[g + G_s]) <= max(8, C_s // 16)
        ):
            G_s += 1
        slabs.append((off, G_s, C_s))
        off += G_s * C_s
        g += G_s
    W = off
    chunk_off = np.empty(G, dtype=np.int64)
    g = 0
    for s_off, G_s, C_s in slabs:
        for k in range(G_s):
            chunk_off[g] = s_off + k * C_s
            g += 1
    return order, deg_sorted, slabs, W, chunk_off


def _order(slabs):
    """Device processing order: smallest slab first (instant pipeline
    start), a mid-size slab second (its exp hides behind the first
    slab's DVE work better than a 6K-column exp would), then descending
    sizes so the kernel ends on the smallest slabs (short tail: the
    final multiplies and output DMAs are tiny)."""
    idx = sorted(range(len(slabs)), key=lambda s: slabs[s][1] * slabs[s][2])
    if len(idx) <= 3:
        return idx
    mid = idx[len(idx) // 2]
    rest = [s for s in idx[:0:-1] if s != mid]
    return [idx[0], mid] + rest


def _mul_split(slabs, proc):
    """Per-slab n_act: the first n_act chunks normalize on ACT (per-chunk
    Copy+scale), the rest in one pair-broadcast DVE tensor_tensor. Greedy
    balance of predicted engine loads; the last two processed slabs stay
    fully on DVE so the ACT queue drains early (short tail)."""
    total_cols = sum(G_s * C_s for _, G_s, C_s in slabs)
    ns = len(slabs)
    act = ns * ACT_EXP[0] + total_cols * ACT_EXP[1]
    # folds (3 levels) + grouped reduce + recip/dup-cast + 2x bmul
    dve = (
        total_cols * (0.5 + 0.25 + 0.125) * DVE_TT[1]
        + total_cols * 0.125 * DVE_RED[1]
        + ns * (3 * DVE_TT[0] + DVE_RED[0] + 2 * DVE_FIX + DVE_BM[0])
        + total_cols * DVE_BM[1]
    )
    n_act = [0] * ns
    late = set(proc[-2:]) if len(proc) > 3 else set()
    idx = [s for s in sorted(range(ns), key=lambda s: -slabs[s][2]) if s not in late]
    moved = True
    while moved and idx:
        moved = False
        for s in idx:
            _, G_s, C_s = slabs[s]
            if n_act[s] >= G_s - 1:
                continue
            d_dve = C_s * DVE_BM[1]
            d_act = ACT_MUL[0] + C_s * ACT_MUL[1]
            if dve - act > d_dve + d_act:
                n_act[s] += 1
                dve -= d_dve
                act += d_act
                moved = True
    return n_act


def _build(slabs, W, q):
    import concourse.mybir as mybir
    from concourse import bacc
    from concourse.tile import TileContext

    nc = bacc.Bacc(None, target_bir_lowering=False)
    ev = nc.dram_tensor("ev", [P, W], mybir.dt.int8, kind="ExternalInput")
    av = nc.dram_tensor("av", [P, W], mybir.dt.float16, kind="ExternalOutput")

    smax = max(S_MAX, max(C for _, _, C in slabs))
    proc = _order(slabs)
    n_acts = _mul_split(slabs, proc)
    with TileContext(nc) as tc:
        with tc.tile_pool(name="sbuf", bufs=4) as pool:

            def load_exp(si):
                """Stage 1: input DMA + slab-wide dequantizing exp."""
                off, G_s, C_s = slabs[si]
                S = G_s * C_s
                et = pool.tile([P, smax], mybir.dt.int8, tag="e")
                nc.sync.dma_start(out=et[:, :S], in_=ev[:, off : off + S])
                xt = pool.tile([P, smax], mybir.dt.float16, tag="x")
                nc.scalar.activation(
                    xt[:, :S],
                    et[:, :S],
                    mybir.ActivationFunctionType.Exp,
                    scale=float(q),
                )
                return xt

            def normalize(si, xt, split_tail=False):
                """Stage 2: fold-adds, grouped reduce, recip, multiplies,
                output DMA. Emitted behind load_exp so the ACT queue
                always has upcoming exps ahead of this slab's per-chunk
                multiplies (keeps the DVE fed). split_tail halves the
                final multiply + output DMA so the last transfer overlaps
                the last compute."""
                (off, G_s, C_s), n_act = slabs[si], n_acts[si]
                S = G_s * C_s
                st = pool.tile([P, G_MAX], mybir.dt.float32, tag="s")
                x3 = xt[:, :S].rearrange("p (g c) -> p g c", g=G_s)
                if C_s % 8 == 0 and C_s >= 32:
                    h, hq, he = C_s // 2, C_s // 4, C_s // 8
                    yt = pool.tile([P, smax // 2], mybir.dt.float16, tag="y")
                    y3 = yt[:, : G_s * h].rearrange("p (g c) -> p g c", g=G_s)
                    nc.vector.tensor_add(out=y3, in0=x3[:, :, :h], in1=x3[:, :, h:])
                    zt = pool.tile([P, smax // 4], mybir.dt.float16, tag="z")
                    z3 = zt[:, : G_s * hq].rearrange("p (g c) -> p g c", g=G_s)
                    nc.vector.tensor_add(out=z3, in0=y3[:, :, :hq], in1=y3[:, :, hq:])
                    wt = pool.tile([P, smax // 8], mybir.dt.float16, tag="w")
                    w3 = wt[:, : G_s * he].rearrange("p (g c) -> p g c", g=G_s)
                    nc.vector.tensor_add(out=w3, in0=z3[:, :, :he], in1=z3[:, :, he:])
                    red_in = w3
                elif C_s % 4 == 0 and C_s >= 8:
                    h, hq = C_s // 2, C_s // 4
                    yt = pool.tile([P, smax // 2], mybir.dt.float16, tag="y")
                    y3 = yt[:, : G_s * h].rearrange("p (g c) -> p g c", g=G_s)
                    nc.vector.tensor_add(out=y3, in0=x3[:, :, :h], in1=x3[:, :, h:])
                    zt = pool.tile([P, smax // 4], mybir.dt.float16, tag="z")
                    z3 = zt[:, : G_s * hq].rearrange("p (g c) -> p g c", g=G_s)
                    nc.vector.tensor_add(out=z3, in0=y3[:, :, :hq], in1=y3[:, :, hq:])
                    red_in = z3
                else:
                    red_in = x3
                nc.vector.tensor_reduce(
                    out=st[:, :G_s],
                    in_=red_in,
                    axis=mybir.AxisListType.X,
                    op=mybir.AluOpType.add,
                )
                qt32 = pool.tile([P, G_MAX], mybir.dt.float32, tag="q32")
                nc.vector.reciprocal(out=qt32[:, :G_s], in_=st[:, :G_s])
                # duplicated-pair fp16 reciprocals: the bmul broadcast AP
                # gets a packed (stride-1, count-2) last dim, which keeps
                # the DVE in 2x mode (a plain stride-0 operand drops to 1x)
                qt2 = pool.tile([P, 2 * G_MAX], mybir.dt.float16, tag="q2")
                with nc.allow_low_precision(reason="1/s fits fp16; gate 2e-2"):
                    nc.vector.tensor_copy(
                        out=qt2[:, : 2 * G_s].rearrange("p (g i) -> p g i", g=G_s),
                        in_=qt32[:, :G_s]
                        .rearrange("p g -> p g ()")
                        .to_broadcast([P, G_s, 2]),
                    )
                at = pool.tile([P, smax], mybir.dt.float16, tag="a")
                for g in range(n_act):
                    o = slice(g * C_s, (g + 1) * C_s)
                    nc.scalar.mul(at[:, o], xt[:, o], qt32[:, g : g + 1])

                def bmul(g0, g1):
                    Gr = g1 - g0
                    o = slice(g0 * C_s, g1 * C_s)
                    nc.vector.tensor_mul(
                        out=at[:, o].rearrange(
                            "p (g c2 i) -> p g c2 i", g=Gr, i=2
                        ),
                        in0=xt[:, o].rearrange(
                            "p (g c2 i) -> p g c2 i", g=Gr, i=2
                        ),
                        in1=qt2[:, 2 * g0 : 2 * g1]
                        .rearrange("p (g i) -> p g () i", g=Gr)
                        .to_broadcast([P, Gr, C_s // 2, 2]),
                    )

                if split_tail and G_s - n_act >= 2:
                    gm = n_act + (G_s - n_act) // 2
                    bmul(n_act, gm)
                    nc.sync.dma_start(
                        out=av[:, off : off + gm * C_s], in_=at[:, : gm * C_s]
                    )
                    bmul(gm, G_s)
                    nc.sync.dma_start(
                        out=av[:, off + gm * C_s : off + S],
                        in_=at[:, gm * C_s : S],
                    )
                else:
                    if n_act < G_s:
                        bmul(n_act, G_s)
                    nc.sync.dma_start(out=av[:, off : off + S], in_=at[:, :S])

            pending = []
            for si in proc:
                xt = load_exp(si)
                pending.append((si, xt))
                if len(pending) > 2:  # 2-deep exp prefetch
                    normalize(*pending.pop(0))
            for k, item in enumerate(pending):
                normalize(*item, split_tail=(k == len(pending) - 1))
    nc.compile()
    return nc


def _prepare(e, tgt, num_nodes):
    """Host-side pack: (per-core int8 arrays, scale, scatter metadata)."""
    E = e.shape[0]
    N = num_nodes
    deg = np.bincount(tgt, minlength=N).astype(np.int64)
    order, deg_sorted, slabs, W, chunk_off = _plan(deg, N)

    q = float(np.abs(e).max()) / 127.0
    e8 = np.clip(np.rint(e * (1.0 / q)), -127, 127).astype(np.int8)

    rankpos = np.empty(N, dtype=np.int64)
    rankpos[order] = np.arange(N, dtype=np.int64)
    r = rankpos[tgt]  # [E] degree-rank of each edge's target
    sidx = np.argsort(r, kind="stable")  # edges grouped by rank
    rs = r[sidx]
    starts = np.concatenate(([0], np.cumsum(deg_sorted[:-1])))
    j = np.arange(E, dtype=np.int64) - starts[rs]  # slot within node
    core = rs % NCORES
    pos = rs // NCORES
    gidx = pos // P
    lane = pos % P
    col = chunk_off[gidx] + j
    flat = lane * W + col

    ev = np.full((NCORES, P * W), -128, dtype=np.int8)
    ev[core, flat] = e8[sidx]
    return ev, slabs, W, q, sidx, core, flat


def kernel(e, edge_index, num_nodes):
    from concourse.bass_utils import run_bass_kernel_spmd

    e = np.ascontiguousarray(np.asarray(e, dtype=np.float32))
    tgt = np.asarray(edge_index)[1].astype(np.int64)
    N = int(num_nodes)
    E = e.shape[0]

    ev, slabs, W, q, sidx, core, flat = _prepare(e, tgt, N)

    key = (tuple(slabs), W, round(q, 9))
    if key not in _CACHE:
        _CACHE[key] = _build(slabs, W, q)
    nc = _CACHE[key]

    in_maps = [{"ev": ev[c].reshape(P, W)} for c in range(NCORES)]
    res = run_bass_kernel_spmd(nc, in_maps, core_ids=list(range(NCORES)))

    av = np.stack([res.results[c]["av"].reshape(-1) for c in range(NCORES)])
    alpha = np.empty(E, dtype=np.float32)
    alpha[sidx] = av[core, flat].astype(np.float32)
    return alpha
